# revision 11
# baseline (speedup 1.0000x reference)
"""ConvLSTM (2 layers, T=16, B=8, 64x64, Hd=64) Trainium2 Bass kernel.

Sharding: data-parallel over batch, one image per NeuronCore (8 cores).
Per core, each timestep's 3x3 SAME conv is computed as 9 shifted fp32r
matmuls accumulating in PSUM (channels on partitions, pixels on the free
dim), gates go through ScalarE (sigmoid/tanh with fused per-partition
bias), and the LSTM cell update runs on VectorE. Recurrent state (h
packed with the conv input, c) stays resident in SBUF for all 16 steps.

Matmul APs must be flat (partition + one contiguous free dim), so conv
inputs are stored width-padded: each 64-pixel row occupies 66 slots
(zero pad columns left/right, plus one guard element at each end of the
tile) at offset y*66; a (dy, dx) shift is then a pure element offset and
every matmul reads/writes one contiguous window. Row clipping handles
dy at the image top/bottom. PSUM holds rows in 512-wide banks (7 or 4
padded rows per bank); the elementwise ops read the interior via strided
APs and keep everything else compact.

Compute-engine ops are partition-aligned (lanes), so the cell update must
keep i, g, f, c, o, tanh(c) on one 64-partition range. Gates are permuted
(via host-side weight-column permutation) so layer 0's chain runs on
partitions 0:64 (where its h must land in comb0/comb1) and layer 1's on
64:128; the single remaining cross-half term (i*g) moves with one small
SBUF->SBUF DMA per row-group.

Layouts (partitions x free):
  comb0 [80, 4226]  = h0 (0:64) | x_t (64:80)      (w0 rows permuted to match)
  comb1 [128, 4226] = h0(t) (0:64) | h1(t-1) (64:128)
  cstate [128, 4096] = c0 (0:64) | c1 (64:128)     (compact)
  layer0 PSUM: ps_A = [f|i], ps_B = [o|g];  layer1: ps_A = [i|f], ps_B = [g|o]
"""
import sys

sys.path.insert(0, "/opt/trn_rl_repo")

import numpy as np

import concourse.bass as bass
import concourse.mybir as mybir
from concourse import bacc
from concourse.tile import TileContext

F32 = mybir.dt.float32
F32R = mybir.dt.float32r
AFT = mybir.ActivationFunctionType

T_STEPS = 16
H = 64
W = 64
HW = H * W  # 4096
WP = W + 2  # padded row stride (66)
COMB_N = H * WP + 2  # 4226: guard elem + 64 padded rows + guard elem
CIN = 16
HD = 64
K0 = CIN + HD  # 80
K1 = HD + HD  # 128

# Interior element (y, x) of a comb tile lives at 1 + y*WP + 1 + x.
INT_OFF = 2

# (0,0) first: it covers every chunk position unclipped, so start=True
# initializes the full PSUM region before the clipped shifts accumulate.
SHIFTS = [(0, 0), (-1, -1), (-1, 0), (-1, 1), (0, -1), (0, 1), (1, -1), (1, 0), (1, 1)]

# PSUM bank chunks: (start row, rows). 7 padded rows = 462 <= 512 (one
# fp32 bank); the last 8 rows split 4+4 so each group is uniform.
CHUNKS = [(0, 7), (7, 7), (14, 7), (21, 7), (28, 7), (35, 7), (42, 7), (49, 7),
          (56, 4), (60, 4)]
# Post-processing groups: two consecutive chunks share one [128, 1024]
# PSUM tile (2 banks).
GROUPS = [(CHUNKS[2 * i], CHUNKS[2 * i + 1]) for i in range(len(CHUNKS) // 2)]

# Gate quarters of the conv output, in reference order: i, f, o, g.
# Per layer: (out-channel order for PSUM tile A, for tile B) in units of
# 64-channel quarters (0=i, 1=f, 2=o, 3=g).
GATE_LAYOUT = {
    0: ((1, 0), (2, 3)),  # ps_A = [f|i], ps_B = [o|g]; cell chain on 0:64
    1: ((0, 1), (3, 2)),  # ps_A = [i|f], ps_B = [g|o]; cell chain on 64:128
}


def _sub_ap(tile_ap, p0, n_p, off, pattern):
    """AP over `tile_ap`'s tensor: partitions p0:p0+n_p, free pattern at
    element offset `off` (pattern = [[step, count], ...])."""
    pstride = tile_ap.ap[0][0]
    return bass.AP(
        tile_ap.tensor, tile_ap.offset + p0 * pstride + off,
        [[pstride, n_p]] + pattern,
    )


def _emit_conv(nc, psum_pool, comb, w_sb, k_lo, k_hi, psum_out):
    """One layer's 3x3 conv for one timestep: gates[256, :] into PSUM.

    comb: width-padded SBUF tile whose partitions k_lo:k_hi hold the input
    channels to contract over. w_sb: [K, 18*128] with column block
    (s*2+mh)*128 holding the [K, 128] transposed weights of shift s, PSUM
    tile mh (A=0, B=1). psum_out collects (group_idx, ps_A, ps_B).
    """
    n_k = k_hi - k_lo
    for gi, group in enumerate(GROUPS):
        ps_pair = []
        for mh in range(2):
            ps = psum_pool.tile([128, 1024], F32)
            for ci, (cs, cn) in enumerate(group):
                for s, (dy, dx) in enumerate(SHIFTS):
                    ys = max(cs, -dy)
                    ye = min(cs + cn, H - dy)
                    nr = ye - ys
                    rhs = _sub_ap(comb[:], k_lo, n_k,
                                  1 + (ys + dy) * WP + dx, [[1, nr * WP]])
                    out = _sub_ap(ps, 0, 128,
                                  ci * 512 + (ys - cs) * WP, [[1, nr * WP]])
                    lhsT = w_sb[k_lo:k_hi,
                                (s * 2 + mh) * 128:(s * 2 + mh + 1) * 128]
                    nc.tensor.matmul(
                        out, lhsT, rhs,
                        start=(s == 0), stop=(s == len(SHIFTS) - 1),
                    )
            ps_pair.append(ps)
        psum_out.append((gi, ps_pair[0], ps_pair[1]))


def _emit_post(nc, post_pool, layer, gi, ps_A, ps_B, b_sb, cstate, h_dests,
               first_step):
    """LSTM elementwise for one row-group (2 PSUM banks, gn rows).

    layer 0: cell chain on partitions 0:64 (c = cstate[0:64]); the i*g
    product forms on 64:128 and is DMA'd down. layer 1: mirrored.
    h_dests: (tile, partition base) pairs receiving h = o*tanh(c) into
    their padded interior rows.
    """
    (cs, cn), (_, cn2) = GROUPS[gi]
    assert cn == cn2
    gn = 2 * cn  # rows in this group
    gbase = cs * 64  # compact element offset of the group's first row

    lo, hi = 0, 64
    p_ch = lo if layer == 0 else hi  # chain half (f, o, c, th, h)
    p_ig = hi if layer == 0 else lo  # product half (i, g)

    # PSUM interior of the 2-bank group: [2 banks, cn rows, 64 cols]
    ps_int = [[512, 2], [WP, cn], [1, 64]]
    # matching compact layout: [2, cn, 64]
    cpk = [[cn * 64, 2], [64, cn], [1, 64]]

    if_sb = post_pool.tile([128, gn * 64], F32, tag="if_sb")
    nc.scalar.activation(
        _sub_ap(if_sb, 0, 128, 0, cpk), _sub_ap(ps_A, 0, 128, 1, ps_int),
        AFT.Sigmoid, bias=b_sb[:, 0:1],
    )
    og_sb = post_pool.tile([128, gn * 64], F32, tag="og_sb")
    nc.scalar.activation(
        _sub_ap(og_sb, p_ch, 64, 0, cpk), _sub_ap(ps_B, p_ch, 64, 1, ps_int),
        AFT.Sigmoid, bias=b_sb[p_ch:p_ch + 64, 1:2],
    )
    nc.scalar.activation(
        _sub_ap(og_sb, p_ig, 64, 0, cpk), _sub_ap(ps_B, p_ig, 64, 1, ps_int),
        AFT.Tanh, bias=b_sb[p_ig:p_ig + 64, 1:2],
    )

    # t1 = i * g on the product half, then DMA across to the chain half.
    t1 = post_pool.tile([128, gn * 64], F32, tag="t1")
    nc.vector.tensor_mul(
        _sub_ap(t1, p_ig, 64, 0, [[1, gn * 64]]),
        _sub_ap(if_sb, p_ig, 64, 0, [[1, gn * 64]]),
        _sub_ap(og_sb, p_ig, 64, 0, [[1, gn * 64]]),
    )
    c_ap = _sub_ap(cstate[:], p_ch, 64, gbase, [[1, gn * 64]])
    if first_step:
        # c was zero: c = i * g, moved directly into the state.
        nc.sync.dma_start(c_ap, _sub_ap(t1, p_ig, 64, 0, [[1, gn * 64]]))
    else:
        t1x = post_pool.tile([128, gn * 64], F32, tag="t1x")
        nc.sync.dma_start(
            _sub_ap(t1x, p_ch, 64, 0, [[1, gn * 64]]),
            _sub_ap(t1, p_ig, 64, 0, [[1, gn * 64]]),
        )
        t2 = post_pool.tile([128, gn * 64], F32, tag="t2")
        nc.vector.tensor_mul(
            _sub_ap(t2, p_ch, 64, 0, [[1, gn * 64]]),
            _sub_ap(if_sb, p_ch, 64, 0, [[1, gn * 64]]),
            c_ap,
        )
        nc.vector.tensor_add(
            c_ap,
            _sub_ap(t2, p_ch, 64, 0, [[1, gn * 64]]),
            _sub_ap(t1x, p_ch, 64, 0, [[1, gn * 64]]),
        )

    th = post_pool.tile([128, gn * 64], F32, tag="th")
    th_ap = _sub_ap(th, p_ch, 64, 0, [[1, gn * 64]])
    nc.scalar.activation(th_ap, c_ap, AFT.Tanh)
    o_ap = _sub_ap(og_sb, p_ch, 64, 0, [[64, gn], [1, 64]])
    th_s = _sub_ap(th, p_ch, 64, 0, [[64, gn], [1, 64]])
    for (dest_tile, dest_p) in h_dests:
        dest = _sub_ap(dest_tile[:], dest_p, 64,
                       INT_OFF + cs * WP, [[WP, gn], [1, 64]])
        nc.vector.tensor_mul(dest, o_ap, th_s)


def _zero_pads(nc, comb, zp, p0, n_p):
    """Zero a comb tile's pad columns and guard elements: positions
    {66k, 66k+1 : k=0..64} are exactly the two guards plus every row's
    left/right pad column. Done via DMA from a zeros tensor (memset can't
    write f32r)."""
    nc.sync.dma_start(
        _sub_ap(comb[:], p0, n_p, 0, [[WP, H + 1], [1, 2]]),
        _sub_ap(zp[:], p0, n_p, 0, [[2, H + 1], [1, 2]]).bitcast(F32R),
    )


def build_program(t_steps=T_STEPS):
    nc = bacc.Bacc("TRN2", target_bir_lowering=False, debug=False, num_devices=8)

    xc = nc.dram_tensor("xc", [t_steps * CIN, COMB_N], F32, kind="ExternalInput")
    zp = nc.dram_tensor("zp", [128, 2 * (H + 1)], F32, kind="ExternalInput")
    w0t = nc.dram_tensor("w0t", [K0, 18 * 128], F32, kind="ExternalInput")
    w1t = nc.dram_tensor("w1t", [K1, 18 * 128], F32, kind="ExternalInput")
    b0t = nc.dram_tensor("b0t", [128, 2], F32, kind="ExternalInput")
    b1t = nc.dram_tensor("b1t", [128, 2], F32, kind="ExternalInput")

    out1 = nc.dram_tensor("out1", [t_steps * HD, HW], F32, kind="ExternalOutput")
    h0f = nc.dram_tensor("h0f", [HD, HW], F32, kind="ExternalOutput")
    c0f = nc.dram_tensor("c0f", [HD, HW], F32, kind="ExternalOutput")
    c1f = nc.dram_tensor("c1f", [HD, HW], F32, kind="ExternalOutput")

    x_dst = [[WP, H], [1, W]]  # interior rows of a comb tile
    x_src = [[W, H], [1, W]]

    with TileContext(nc) as tc:
        with (
            tc.tile_pool(name="const", bufs=1) as const_pool,
            tc.tile_pool(name="comb0", bufs=2) as comb0_pool,
            tc.tile_pool(name="comb1", bufs=2) as comb1_pool,
            tc.tile_pool(name="state", bufs=1) as state_pool,
            tc.tile_pool(name="post", bufs=2) as post_pool,
            tc.tile_pool(name="psum", bufs=4, space="PSUM") as psum_pool,
        ):
            w0_sb = const_pool.tile([K0, 18 * 128], F32R, tag="w0")
            nc.sync.dma_start(w0_sb[:], w0t[:].bitcast(F32R))
            w1_sb = const_pool.tile([K1, 18 * 128], F32R, tag="w1")
            nc.sync.dma_start(w1_sb[:], w1t[:].bitcast(F32R))
            b0_sb = const_pool.tile([128, 2], F32, tag="b0")
            nc.sync.dma_start(b0_sb[:], b0t[:])
            b1_sb = const_pool.tile([128, 2], F32, tag="b1")
            nc.sync.dma_start(b1_sb[:], b1t[:])

            # c0 on partitions 0:64, c1 on 64:128 (compact layout)
            cstate = state_pool.tile([128, HW], F32, tag="cstate")

            comb0_cur = comb0_pool.tile([K0, COMB_N], F32R, tag="comb0")
            _zero_pads(nc, comb0_cur, zp, 0, HD)
            nc.sync.dma_start(
                _sub_ap(comb0_cur[:], HD, CIN, 0, [[1, COMB_N]]),
                _sub_ap(xc[:], 0, CIN, 0, [[1, COMB_N]]).bitcast(F32R),
            )
            comb1_cur = comb1_pool.tile([K1, COMB_N], F32R, tag="comb1")
            _zero_pads(nc, comb1_cur, zp, 0, K1)

            for t in range(t_steps):
                comb0_next = comb0_pool.tile([K0, COMB_N], F32R, tag="comb0")
                _zero_pads(nc, comb0_next, zp, 0, HD)
                if t + 1 < t_steps:
                    nc.sync.dma_start(
                        _sub_ap(comb0_next[:], HD, CIN, 0, [[1, COMB_N]]),
                        _sub_ap(xc[:], (t + 1) * CIN, CIN, 0,
                                [[1, COMB_N]]).bitcast(F32R),
                    )
                else:
                    # no x for t+1: still zero the x half's pads (cheap, and
                    # keeps the final-state tile fully defined)
                    _zero_pads(nc, comb0_next, zp, HD, CIN)
                comb1_next = comb1_pool.tile([K1, COMB_N], F32R, tag="comb1")
                _zero_pads(nc, comb1_next, zp, 0, K1)

                # ---- layer 0 ----
                # h0(-1) == 0: at t=0 contract only over the x channels,
                # which sit at partitions 64:80.
                k_lo0 = HD if t == 0 else 0
                groups0 = []
                _emit_conv(nc, psum_pool, comb0_cur, w0_sb, k_lo0, K0, groups0)
                for (gi, ps_A, ps_B) in groups0:
                    _emit_post(
                        nc, post_pool, 0, gi, ps_A, ps_B, b0_sb, cstate,
                        [(comb0_next, 0), (comb1_cur, 0)],
                        first_step=(t == 0),
                    )

                # ---- layer 1 ----
                k_hi1 = HD if t == 0 else K1  # h1(-1) == 0: skip 64:128
                groups1 = []
                _emit_conv(nc, psum_pool, comb1_cur, w1_sb, 0, k_hi1, groups1)
                for (gi, ps_A, ps_B) in groups1:
                    _emit_post(
                        nc, post_pool, 1, gi, ps_A, ps_B, b1_sb, cstate,
                        [(comb1_next, HD)],
                        first_step=(t == 0),
                    )

                nc.sync.dma_start(
                    _sub_ap(out1[:], t * HD, HD, 0, x_src),
                    _sub_ap(comb1_next[:], HD, HD, INT_OFF, x_dst).bitcast(F32),
                )

                comb0_cur = comb0_next
                comb1_cur = comb1_next

            nc.sync.dma_start(
                _sub_ap(h0f[:], 0, HD, 0, x_src),
                _sub_ap(comb0_cur[:], 0, HD, INT_OFF, x_dst).bitcast(F32),
            )
            nc.sync.dma_start(c0f[:], cstate[0:HD, :])
            nc.sync.dma_start(c1f[:], cstate[HD:128, :])

    nc.compile()
    return nc


def _gate_perm(layer):
    """256-entry output-channel order: [tile A quarters, tile B quarters]."""
    (a0, a1), (b0, b1) = GATE_LAYOUT[layer]
    order = []
    for q in (a0, a1, b0, b1):
        order.extend(range(q * 64, (q + 1) * 64))
    return order


def _prep_weights(w, K, layer):
    """w [256, K, 3, 3] -> [K, 18*128] lhsT blocks per (shift, psum tile).

    Output channels are permuted per GATE_LAYOUT. For layer 0 the
    input-channel rows are also permuted to the on-chip comb0 layout
    [h(64) | x(16)] (reference concat order is [x, h])."""
    w = np.asarray(w, np.float32)[_gate_perm(layer)]
    if layer == 0:
        assert K == K0
        perm = list(range(CIN, K0)) + list(range(CIN))
        w = w[:, perm]
    out = np.empty((K, 18, 128), np.float32)
    for s, (dy, dx) in enumerate(SHIFTS):
        for mh in range(2):
            out[:, s * 2 + mh, :] = w[mh * 128:(mh + 1) * 128, :, dy + 1, dx + 1].T
    return np.ascontiguousarray(out.reshape(K, 18 * 128))


def _prep_bias(b, layer):
    """b [256] -> [128, 2]: col 0 = tile A bias, col 1 = tile B bias."""
    bp = np.asarray(b, np.float32)[_gate_perm(layer)]
    return np.ascontiguousarray(bp.reshape(2, 128).T)


_NC_CACHE = {}


def kernel(x, w0, b0, w1, b1):
    from concourse.bass_utils import run_bass_kernel_spmd

    x = np.ascontiguousarray(np.asarray(x), dtype=np.float32)
    B, T = x.shape[0], x.shape[1]
    assert (B, T) == (8, T_STEPS) and x.shape[2:] == (CIN, H, W)

    w0t = _prep_weights(np.asarray(w0, dtype=np.float32), K0, 0)
    w1t = _prep_weights(np.asarray(w1, dtype=np.float32), K1, 1)
    b0t = _prep_bias(b0, 0)
    b1t = _prep_bias(b1, 1)

    xs = np.zeros((B, T * CIN, COMB_N), np.float32)
    xs[:, :, 1:1 + H * WP].reshape(B, T * CIN, H, WP)[:, :, :, 1:1 + W] = (
        x.reshape(B, T * CIN, H, W)
    )
    zp = np.zeros((128, 2 * (H + 1)), np.float32)
    in_maps = [
        {"xc": xs[i], "w0t": w0t, "w1t": w1t, "b0t": b0t, "b1t": b1t, "zp": zp}
        for i in range(B)
    ]

    if "nc" not in _NC_CACHE:
        _NC_CACHE["nc"] = build_program()
    nc = _NC_CACHE["nc"]

    res = run_bass_kernel_spmd(nc, in_maps, core_ids=list(range(8)), trace=False)

    out1 = np.stack(
        [res.results[i]["out1"].reshape(T_STEPS, HD, H, W) for i in range(B)]
    )
    h0 = np.stack([res.results[i]["h0f"].reshape(HD, H, W) for i in range(B)])
    c0 = np.stack([res.results[i]["c0f"].reshape(HD, H, W) for i in range(B)])
    c1 = np.stack([res.results[i]["c1f"].reshape(HD, H, W) for i in range(B)])
    h1 = np.ascontiguousarray(out1[:, -1])
    return out1, h0, c0, h1, c1


# revision 12
# speedup vs baseline: 1.0027x; 1.0027x over previous
"""ConvLSTM (2 layers, T=16, B=8, 64x64, Hd=64) Trainium2 Bass kernel.

Sharding: data-parallel over batch, one image per NeuronCore (8 cores).
Per core, each timestep's 3x3 SAME conv is computed as 9 shifted fp32r
matmuls accumulating in PSUM (channels on partitions, pixels on the free
dim), gates go through ScalarE (sigmoid/tanh with fused per-partition
bias), and the LSTM cell update runs on VectorE. Recurrent state (h
packed with the conv input, c) stays resident in SBUF for all 16 steps.

Matmul APs must be flat (partition + one contiguous free dim), so conv
inputs are stored width-padded: each 64-pixel row occupies 66 slots
(zero pad columns left/right, plus one guard element at each end of the
tile) at offset y*66; a (dy, dx) shift is then a pure element offset and
every matmul reads/writes one contiguous window. Row clipping handles
dy at the image top/bottom. PSUM holds rows in 512-wide banks (7 or 4
padded rows per bank); the elementwise ops read the interior via strided
APs and keep everything else compact.

Compute-engine ops are partition-aligned (lanes), so the cell update must
keep i, g, f, c, o, tanh(c) on one 64-partition range. Gates are permuted
(via host-side weight-column permutation) so layer 0's chain runs on
partitions 0:64 (where its h must land in comb0/comb1) and layer 1's on
64:128; the single remaining cross-half term (i*g) moves with one small
SBUF->SBUF DMA per row-group.

Layouts (partitions x free):
  comb0 [80, 4226]  = h0 (0:64) | x_t (64:80)      (w0 rows permuted to match)
  comb1 [128, 4226] = h0(t) (0:64) | h1(t-1) (64:128)
  cstate [128, 4096] = c0 (0:64) | c1 (64:128)     (compact)
  layer0 PSUM: ps_A = [f|i], ps_B = [o|g];  layer1: ps_A = [i|f], ps_B = [g|o]
"""
import os
import sys

sys.path.insert(0, "/opt/trn_rl_repo")

# The kernel must run on the axon-tunneled NeuronCores; drop a platform
# pin (e.g. JAX_PLATFORMS=cpu meant for the reference) that would mask it.
if "axon" not in os.environ.get("JAX_PLATFORMS", "axon"):
    os.environ.pop("JAX_PLATFORMS", None)

import numpy as np

import concourse.bass as bass
import concourse.mybir as mybir
from concourse import bacc
from concourse.tile import TileContext

F32 = mybir.dt.float32
F32R = mybir.dt.float32r
AFT = mybir.ActivationFunctionType

T_STEPS = 16
H = 64
W = 64
HW = H * W  # 4096
WP = W + 2  # padded row stride (66)
COMB_N = H * WP + 2  # 4226: guard elem + 64 padded rows + guard elem
CIN = 16
HD = 64
K0 = CIN + HD  # 80
K1 = HD + HD  # 128

# Interior element (y, x) of a comb tile lives at 1 + y*WP + 1 + x.
INT_OFF = 2

# (0,0) first: it covers every chunk position unclipped, so start=True
# initializes the full PSUM region before the clipped shifts accumulate.
SHIFTS = [(0, 0), (-1, -1), (-1, 0), (-1, 1), (0, -1), (0, 1), (1, -1), (1, 0), (1, 1)]

# PSUM bank chunks: (start row, rows). 7 padded rows = 462 <= 512 (one
# fp32 bank); the last 8 rows split 4+4 so each group is uniform.
CHUNKS = [(0, 7), (7, 7), (14, 7), (21, 7), (28, 7), (35, 7), (42, 7), (49, 7),
          (56, 4), (60, 4)]
# Post-processing groups: two consecutive chunks share one [128, 1024]
# PSUM tile (2 banks).
GROUPS = [(CHUNKS[2 * i], CHUNKS[2 * i + 1]) for i in range(len(CHUNKS) // 2)]

# Gate quarters of the conv output, in reference order: i, f, o, g.
# Per layer: (out-channel order for PSUM tile A, for tile B) in units of
# 64-channel quarters (0=i, 1=f, 2=o, 3=g).
GATE_LAYOUT = {
    0: ((1, 0), (2, 3)),  # ps_A = [f|i], ps_B = [o|g]; cell chain on 0:64
    1: ((0, 1), (3, 2)),  # ps_A = [i|f], ps_B = [g|o]; cell chain on 64:128
}


def _sub_ap(tile_ap, p0, n_p, off, pattern):
    """AP over `tile_ap`'s tensor: partitions p0:p0+n_p, free pattern at
    element offset `off` (pattern = [[step, count], ...])."""
    pstride = tile_ap.ap[0][0]
    return bass.AP(
        tile_ap.tensor, tile_ap.offset + p0 * pstride + off,
        [[pstride, n_p]] + pattern,
    )


def _emit_conv(nc, psum_pool, comb, w_sb, k_lo, k_hi, psum_out):
    """One layer's 3x3 conv for one timestep: gates[256, :] into PSUM.

    comb: width-padded SBUF tile whose partitions k_lo:k_hi hold the input
    channels to contract over. w_sb: [K, 18*128] with column block
    (s*2+mh)*128 holding the [K, 128] transposed weights of shift s, PSUM
    tile mh (A=0, B=1). psum_out collects (group_idx, ps_A, ps_B).
    """
    n_k = k_hi - k_lo
    for gi, group in enumerate(GROUPS):
        ps_pair = []
        for mh in range(2):
            ps = psum_pool.tile([128, 1024], F32)
            for ci, (cs, cn) in enumerate(group):
                for s, (dy, dx) in enumerate(SHIFTS):
                    ys = max(cs, -dy)
                    ye = min(cs + cn, H - dy)
                    nr = ye - ys
                    rhs = _sub_ap(comb[:], k_lo, n_k,
                                  1 + (ys + dy) * WP + dx, [[1, nr * WP]])
                    out = _sub_ap(ps, 0, 128,
                                  ci * 512 + (ys - cs) * WP, [[1, nr * WP]])
                    lhsT = w_sb[k_lo:k_hi,
                                (s * 2 + mh) * 128:(s * 2 + mh + 1) * 128]
                    nc.tensor.matmul(
                        out, lhsT, rhs,
                        start=(s == 0), stop=(s == len(SHIFTS) - 1),
                    )
            ps_pair.append(ps)
        psum_out.append((gi, ps_pair[0], ps_pair[1]))


def _emit_post(nc, post_pool, layer, gi, ps_A, ps_B, b_sb, cstate, h_dests,
               first_step):
    """LSTM elementwise for one row-group (2 PSUM banks, gn rows).

    layer 0: cell chain on partitions 0:64 (c = cstate[0:64]); the i*g
    product forms on 64:128 and is DMA'd down. layer 1: mirrored.
    h_dests: (tile, partition base) pairs receiving h = o*tanh(c) into
    their padded interior rows.
    """
    (cs, cn), (_, cn2) = GROUPS[gi]
    assert cn == cn2
    gn = 2 * cn  # rows in this group
    gbase = cs * 64  # compact element offset of the group's first row

    lo, hi = 0, 64
    p_ch = lo if layer == 0 else hi  # chain half (f, o, c, th, h)
    p_ig = hi if layer == 0 else lo  # product half (i, g)

    # PSUM interior of the 2-bank group: [2 banks, cn rows, 64 cols]
    ps_int = [[512, 2], [WP, cn], [1, 64]]
    # matching compact layout: [2, cn, 64]
    cpk = [[cn * 64, 2], [64, cn], [1, 64]]

    if_sb = post_pool.tile([128, gn * 64], F32, tag="if_sb")
    nc.scalar.activation(
        _sub_ap(if_sb, 0, 128, 0, cpk), _sub_ap(ps_A, 0, 128, 1, ps_int),
        AFT.Sigmoid, bias=b_sb[:, 0:1],
    )
    og_sb = post_pool.tile([128, gn * 64], F32, tag="og_sb")
    nc.scalar.activation(
        _sub_ap(og_sb, p_ch, 64, 0, cpk), _sub_ap(ps_B, p_ch, 64, 1, ps_int),
        AFT.Sigmoid, bias=b_sb[p_ch:p_ch + 64, 1:2],
    )
    nc.scalar.activation(
        _sub_ap(og_sb, p_ig, 64, 0, cpk), _sub_ap(ps_B, p_ig, 64, 1, ps_int),
        AFT.Tanh, bias=b_sb[p_ig:p_ig + 64, 1:2],
    )

    # t1 = i * g on the product half, then DMA across to the chain half.
    t1 = post_pool.tile([128, gn * 64], F32, tag="t1")
    nc.vector.tensor_mul(
        _sub_ap(t1, p_ig, 64, 0, [[1, gn * 64]]),
        _sub_ap(if_sb, p_ig, 64, 0, [[1, gn * 64]]),
        _sub_ap(og_sb, p_ig, 64, 0, [[1, gn * 64]]),
    )
    c_ap = _sub_ap(cstate[:], p_ch, 64, gbase, [[1, gn * 64]])
    if first_step:
        # c was zero: c = i * g, moved directly into the state.
        nc.sync.dma_start(c_ap, _sub_ap(t1, p_ig, 64, 0, [[1, gn * 64]]))
    else:
        t1x = post_pool.tile([128, gn * 64], F32, tag="t1x")
        nc.sync.dma_start(
            _sub_ap(t1x, p_ch, 64, 0, [[1, gn * 64]]),
            _sub_ap(t1, p_ig, 64, 0, [[1, gn * 64]]),
        )
        t2 = post_pool.tile([128, gn * 64], F32, tag="t2")
        nc.vector.tensor_mul(
            _sub_ap(t2, p_ch, 64, 0, [[1, gn * 64]]),
            _sub_ap(if_sb, p_ch, 64, 0, [[1, gn * 64]]),
            c_ap,
        )
        nc.vector.tensor_add(
            c_ap,
            _sub_ap(t2, p_ch, 64, 0, [[1, gn * 64]]),
            _sub_ap(t1x, p_ch, 64, 0, [[1, gn * 64]]),
        )

    th = post_pool.tile([128, gn * 64], F32, tag="th")
    th_ap = _sub_ap(th, p_ch, 64, 0, [[1, gn * 64]])
    nc.scalar.activation(th_ap, c_ap, AFT.Tanh)
    o_ap = _sub_ap(og_sb, p_ch, 64, 0, [[64, gn], [1, 64]])
    th_s = _sub_ap(th, p_ch, 64, 0, [[64, gn], [1, 64]])
    for (dest_tile, dest_p) in h_dests:
        dest = _sub_ap(dest_tile[:], dest_p, 64,
                       INT_OFF + cs * WP, [[WP, gn], [1, 64]])
        nc.vector.tensor_mul(dest, o_ap, th_s)


def _zero_pads(nc, comb, zp, p0, n_p):
    """Zero a comb tile's pad columns and guard elements: positions
    {66k, 66k+1 : k=0..64} are exactly the two guards plus every row's
    left/right pad column. Done via DMA from a zeros tensor (memset can't
    write f32r)."""
    nc.sync.dma_start(
        _sub_ap(comb[:], p0, n_p, 0, [[WP, H + 1], [1, 2]]),
        _sub_ap(zp[:], p0, n_p, 0, [[2, H + 1], [1, 2]]).bitcast(F32R),
    )


def build_program(t_steps=T_STEPS):
    nc = bacc.Bacc("TRN2", target_bir_lowering=False, debug=False, num_devices=8)

    xc = nc.dram_tensor("xc", [t_steps * CIN, COMB_N], F32, kind="ExternalInput")
    zp = nc.dram_tensor("zp", [128, 2 * (H + 1)], F32, kind="ExternalInput")
    w0t = nc.dram_tensor("w0t", [K0, 18 * 128], F32, kind="ExternalInput")
    w1t = nc.dram_tensor("w1t", [K1, 18 * 128], F32, kind="ExternalInput")
    b0t = nc.dram_tensor("b0t", [128, 2], F32, kind="ExternalInput")
    b1t = nc.dram_tensor("b1t", [128, 2], F32, kind="ExternalInput")

    out1 = nc.dram_tensor("out1", [t_steps * HD, HW], F32, kind="ExternalOutput")
    h0f = nc.dram_tensor("h0f", [HD, HW], F32, kind="ExternalOutput")
    c0f = nc.dram_tensor("c0f", [HD, HW], F32, kind="ExternalOutput")
    c1f = nc.dram_tensor("c1f", [HD, HW], F32, kind="ExternalOutput")

    x_dst = [[WP, H], [1, W]]  # interior rows of a comb tile
    x_src = [[W, H], [1, W]]

    with TileContext(nc) as tc:
        with (
            tc.tile_pool(name="const", bufs=1) as const_pool,
            tc.tile_pool(name="comb0", bufs=2) as comb0_pool,
            tc.tile_pool(name="comb1", bufs=2) as comb1_pool,
            tc.tile_pool(name="state", bufs=1) as state_pool,
            tc.tile_pool(name="post", bufs=2) as post_pool,
            tc.tile_pool(name="psum", bufs=4, space="PSUM") as psum_pool,
        ):
            # First conv needs w0 + x(0) + comb0 pads; load those first so
            # the PE starts as early as possible, then the rest.
            w0_sb = const_pool.tile([K0, 18 * 128], F32R, tag="w0")
            nc.sync.dma_start(w0_sb[:], w0t[:].bitcast(F32R))
            b0_sb = const_pool.tile([128, 2], F32, tag="b0")
            nc.sync.dma_start(b0_sb[:], b0t[:])
            comb0_cur = comb0_pool.tile([K0, COMB_N], F32R, tag="comb0")
            _zero_pads(nc, comb0_cur, zp, 0, HD)
            nc.sync.dma_start(
                _sub_ap(comb0_cur[:], HD, CIN, 0, [[1, COMB_N]]),
                _sub_ap(xc[:], 0, CIN, 0, [[1, COMB_N]]).bitcast(F32R),
            )
            w1_sb = const_pool.tile([K1, 18 * 128], F32R, tag="w1")
            nc.sync.dma_start(w1_sb[:], w1t[:].bitcast(F32R))
            b1_sb = const_pool.tile([128, 2], F32, tag="b1")
            nc.sync.dma_start(b1_sb[:], b1t[:])

            # c0 on partitions 0:64, c1 on 64:128 (compact layout)
            cstate = state_pool.tile([128, HW], F32, tag="cstate")

            comb1_cur = comb1_pool.tile([K1, COMB_N], F32R, tag="comb1")
            _zero_pads(nc, comb1_cur, zp, 0, K1)

            for t in range(t_steps):
                comb0_next = comb0_pool.tile([K0, COMB_N], F32R, tag="comb0")
                _zero_pads(nc, comb0_next, zp, 0, HD)
                if t + 1 < t_steps:
                    nc.sync.dma_start(
                        _sub_ap(comb0_next[:], HD, CIN, 0, [[1, COMB_N]]),
                        _sub_ap(xc[:], (t + 1) * CIN, CIN, 0,
                                [[1, COMB_N]]).bitcast(F32R),
                    )
                else:
                    # no x for t+1: still zero the x half's pads (cheap, and
                    # keeps the final-state tile fully defined)
                    _zero_pads(nc, comb0_next, zp, HD, CIN)
                comb1_next = comb1_pool.tile([K1, COMB_N], F32R, tag="comb1")
                _zero_pads(nc, comb1_next, zp, 0, K1)

                # ---- layer 0 ----
                # h0(-1) == 0: at t=0 contract only over the x channels,
                # which sit at partitions 64:80.
                k_lo0 = HD if t == 0 else 0
                groups0 = []
                _emit_conv(nc, psum_pool, comb0_cur, w0_sb, k_lo0, K0, groups0)
                for (gi, ps_A, ps_B) in groups0:
                    _emit_post(
                        nc, post_pool, 0, gi, ps_A, ps_B, b0_sb, cstate,
                        [(comb0_next, 0), (comb1_cur, 0)],
                        first_step=(t == 0),
                    )

                if t == t_steps - 1:
                    # h0(15)/c0 are final after layer 0's last posts; drain
                    # them to HBM under layer 1's compute.
                    nc.sync.dma_start(
                        _sub_ap(h0f[:], 0, HD, 0, x_src),
                        _sub_ap(comb0_next[:], 0, HD, INT_OFF, x_dst).bitcast(F32),
                    )
                    nc.sync.dma_start(c0f[:], cstate[0:HD, :])

                # ---- layer 1 ----
                k_hi1 = HD if t == 0 else K1  # h1(-1) == 0: skip 64:128
                groups1 = []
                _emit_conv(nc, psum_pool, comb1_cur, w1_sb, 0, k_hi1, groups1)
                for (gi, ps_A, ps_B) in groups1:
                    _emit_post(
                        nc, post_pool, 1, gi, ps_A, ps_B, b1_sb, cstate,
                        [(comb1_next, HD)],
                        first_step=(t == 0),
                    )

                nc.sync.dma_start(
                    _sub_ap(out1[:], t * HD, HD, 0, x_src),
                    _sub_ap(comb1_next[:], HD, HD, INT_OFF, x_dst).bitcast(F32),
                )

                comb0_cur = comb0_next
                comb1_cur = comb1_next

            nc.sync.dma_start(c1f[:], cstate[HD:128, :])

    nc.compile()
    return nc


def _gate_perm(layer):
    """256-entry output-channel order: [tile A quarters, tile B quarters]."""
    (a0, a1), (b0, b1) = GATE_LAYOUT[layer]
    order = []
    for q in (a0, a1, b0, b1):
        order.extend(range(q * 64, (q + 1) * 64))
    return order


def _prep_weights(w, K, layer):
    """w [256, K, 3, 3] -> [K, 18*128] lhsT blocks per (shift, psum tile).

    Output channels are permuted per GATE_LAYOUT. For layer 0 the
    input-channel rows are also permuted to the on-chip comb0 layout
    [h(64) | x(16)] (reference concat order is [x, h])."""
    w = np.asarray(w, np.float32)[_gate_perm(layer)]
    if layer == 0:
        assert K == K0
        perm = list(range(CIN, K0)) + list(range(CIN))
        w = w[:, perm]
    out = np.empty((K, 18, 128), np.float32)
    for s, (dy, dx) in enumerate(SHIFTS):
        for mh in range(2):
            out[:, s * 2 + mh, :] = w[mh * 128:(mh + 1) * 128, :, dy + 1, dx + 1].T
    return np.ascontiguousarray(out.reshape(K, 18 * 128))


def _prep_bias(b, layer):
    """b [256] -> [128, 2]: col 0 = tile A bias, col 1 = tile B bias."""
    bp = np.asarray(b, np.float32)[_gate_perm(layer)]
    return np.ascontiguousarray(bp.reshape(2, 128).T)


_NC_CACHE = {}


def kernel(x, w0, b0, w1, b1):
    from concourse.bass_utils import run_bass_kernel_spmd

    x = np.ascontiguousarray(np.asarray(x), dtype=np.float32)
    B, T = x.shape[0], x.shape[1]
    assert (B, T) == (8, T_STEPS) and x.shape[2:] == (CIN, H, W)

    w0t = _prep_weights(np.asarray(w0, dtype=np.float32), K0, 0)
    w1t = _prep_weights(np.asarray(w1, dtype=np.float32), K1, 1)
    b0t = _prep_bias(b0, 0)
    b1t = _prep_bias(b1, 1)

    xs = np.zeros((B, T * CIN, COMB_N), np.float32)
    xs[:, :, 1:1 + H * WP].reshape(B, T * CIN, H, WP)[:, :, :, 1:1 + W] = (
        x.reshape(B, T * CIN, H, W)
    )
    zp = np.zeros((128, 2 * (H + 1)), np.float32)
    in_maps = [
        {"xc": xs[i], "w0t": w0t, "w1t": w1t, "b0t": b0t, "b1t": b1t, "zp": zp}
        for i in range(B)
    ]

    if "nc" not in _NC_CACHE:
        _NC_CACHE["nc"] = build_program()
    nc = _NC_CACHE["nc"]

    res = run_bass_kernel_spmd(nc, in_maps, core_ids=list(range(8)), trace=False)

    out1 = np.stack(
        [res.results[i]["out1"].reshape(T_STEPS, HD, H, W) for i in range(B)]
    )
    h0 = np.stack([res.results[i]["h0f"].reshape(HD, H, W) for i in range(B)])
    c0 = np.stack([res.results[i]["c0f"].reshape(HD, H, W) for i in range(B)])
    c1 = np.stack([res.results[i]["c1f"].reshape(HD, H, W) for i in range(B)])
    h1 = np.ascontiguousarray(out1[:, -1])
    return out1, h0, c0, h1, c1


# revision 14
# speedup vs baseline: 1.0103x; 1.0076x over previous
"""ConvLSTM (2 layers, T=16, B=8, 64x64, Hd=64) Trainium2 Bass kernel.

Sharding: data-parallel over batch, one image per NeuronCore (8 cores).
Per core, each timestep's 3x3 SAME conv is computed as 9 shifted fp32r
matmuls accumulating in PSUM (channels on partitions, pixels on the free
dim), gates go through ScalarE (sigmoid/tanh with fused per-partition
bias), and the LSTM cell update runs on VectorE. Recurrent state (h
packed with the conv input, c) stays resident in SBUF for all 16 steps.

Matmul APs must be flat (partition + one contiguous free dim), so conv
inputs are stored width-padded: each 64-pixel row occupies 66 slots
(zero pad columns left/right, plus one guard element at each end of the
tile) at offset y*66; a (dy, dx) shift is then a pure element offset and
every matmul reads/writes one contiguous window. Row clipping handles
dy at the image top/bottom. PSUM holds rows in 512-wide banks (7 or 4
padded rows per bank); the elementwise ops read the interior via strided
APs and keep everything else compact.

Compute-engine ops are partition-aligned (lanes), so the cell update must
keep i, g, f, c, o, tanh(c) on one 64-partition range. Gates are permuted
(via host-side weight-column permutation) so layer 0's chain runs on
partitions 0:64 (where its h must land in comb0/comb1) and layer 1's on
64:128; the single remaining cross-half term (i*g) moves with one small
SBUF->SBUF DMA per row-group.

Layouts (partitions x free):
  comb0 [80, 4226]  = h0 (0:64) | x_t (64:80)      (w0 rows permuted to match)
  comb1 [128, 4226] = h0(t) (0:64) | h1(t-1) (64:128)
  cstate [128, 4096] = c0 (0:64) | c1 (64:128)     (compact)
  layer0 PSUM: ps_A = [f|i], ps_B = [o|g];  layer1: ps_A = [i|f], ps_B = [g|o]
"""
import os
import sys

sys.path.insert(0, "/opt/trn_rl_repo")

# The kernel must run on the axon-tunneled NeuronCores; drop a platform
# pin (e.g. JAX_PLATFORMS=cpu meant for the reference) that would mask it.
if "axon" not in os.environ.get("JAX_PLATFORMS", "axon"):
    os.environ.pop("JAX_PLATFORMS", None)

import numpy as np

import concourse.bass as bass
import concourse.mybir as mybir
from concourse import bacc
from concourse.tile import TileContext

F32 = mybir.dt.float32
F32R = mybir.dt.float32r
AFT = mybir.ActivationFunctionType

T_STEPS = 16
H = 64
W = 64
HW = H * W  # 4096
WP = W + 2  # padded row stride (66)
COMB_N = H * WP + 2  # 4226: guard elem + 64 padded rows + guard elem
CIN = 16
HD = 64
K0 = CIN + HD  # 80
K1 = HD + HD  # 128

# Interior element (y, x) of a comb tile lives at 1 + y*WP + 1 + x.
INT_OFF = 2

# (0,0) first: it covers every chunk position unclipped, so start=True
# initializes the full PSUM region before the clipped shifts accumulate.
SHIFTS = [(0, 0), (-1, -1), (-1, 0), (-1, 1), (0, -1), (0, 1), (1, -1), (1, 0), (1, 1)]

# PSUM bank chunks: (start row, rows). 7 padded rows = 462 <= 512 (one
# fp32 bank); the last 8 rows split 4+4 so each group is uniform.
CHUNKS = [(0, 7), (7, 7), (14, 7), (21, 7), (28, 7), (35, 7), (42, 7), (49, 7),
          (56, 4), (60, 4)]
# Post-processing groups: two consecutive chunks share one [128, 1024]
# PSUM tile (2 banks).
GROUPS = [(CHUNKS[2 * i], CHUNKS[2 * i + 1]) for i in range(len(CHUNKS) // 2)]

# Gate quarters of the conv output, in reference order: i, f, o, g.
# Per layer: (out-channel order for PSUM tile A, for tile B) in units of
# 64-channel quarters (0=i, 1=f, 2=o, 3=g).
GATE_LAYOUT = {
    0: ((1, 0), (2, 3)),  # ps_A = [f|i], ps_B = [o|g]; cell chain on 0:64
    1: ((0, 1), (3, 2)),  # ps_A = [i|f], ps_B = [g|o]; cell chain on 64:128
}


def _sub_ap(tile_ap, p0, n_p, off, pattern):
    """AP over `tile_ap`'s tensor: partitions p0:p0+n_p, free pattern at
    element offset `off` (pattern = [[step, count], ...])."""
    pstride = tile_ap.ap[0][0]
    return bass.AP(
        tile_ap.tensor, tile_ap.offset + p0 * pstride + off,
        [[pstride, n_p]] + pattern,
    )


def _emit_conv(nc, psum_pool, comb, w_sb, k_lo, k_hi, psum_out):
    """One layer's 3x3 conv for one timestep: gates[256, :] into PSUM.

    comb: width-padded SBUF tile whose partitions k_lo:k_hi hold the input
    channels to contract over. w_sb: [K, 18*128] with column block
    (s*2+mh)*128 holding the [K, 128] transposed weights of shift s, PSUM
    tile mh (A=0, B=1). psum_out collects (group_idx, ps_A, ps_B).
    """
    n_k = k_hi - k_lo
    for gi, group in enumerate(GROUPS):
        ps_pair = []
        for mh in range(2):
            ps = psum_pool.tile([128, 1024], F32)
            for ci, (cs, cn) in enumerate(group):
                for s, (dy, dx) in enumerate(SHIFTS):
                    ys = max(cs, -dy)
                    ye = min(cs + cn, H - dy)
                    nr = ye - ys
                    rhs = _sub_ap(comb[:], k_lo, n_k,
                                  1 + (ys + dy) * WP + dx, [[1, nr * WP]])
                    out = _sub_ap(ps, 0, 128,
                                  ci * 512 + (ys - cs) * WP, [[1, nr * WP]])
                    lhsT = w_sb[k_lo:k_hi,
                                (s * 2 + mh) * 128:(s * 2 + mh + 1) * 128]
                    nc.tensor.matmul(
                        out, lhsT, rhs,
                        start=(s == 0), stop=(s == len(SHIFTS) - 1),
                    )
            ps_pair.append(ps)
        psum_out.append((gi, ps_pair[0], ps_pair[1]))


def _emit_post(nc, post_pool, layer, gi, ps_A, ps_B, b_sb, cstate, h_dests,
               first_step):
    """LSTM elementwise for one row-group (2 PSUM banks, gn rows).

    layer 0: cell chain on partitions 0:64 (c = cstate[0:64]); the i*g
    product forms on 64:128 and is DMA'd down. layer 1: mirrored.
    h_dests: (tile, partition base) pairs receiving h = o*tanh(c) into
    their padded interior rows.
    """
    (cs, cn), (_, cn2) = GROUPS[gi]
    assert cn == cn2
    gn = 2 * cn  # rows in this group
    gbase = cs * 64  # compact element offset of the group's first row

    lo, hi = 0, 64
    p_ch = lo if layer == 0 else hi  # chain half (f, o, c, th, h)
    p_ig = hi if layer == 0 else lo  # product half (i, g)

    # PSUM interior of the 2-bank group: [2 banks, cn rows, 64 cols]
    ps_int = [[512, 2], [WP, cn], [1, 64]]
    # matching compact layout: [2, cn, 64]
    cpk = [[cn * 64, 2], [64, cn], [1, 64]]

    if_sb = post_pool.tile([128, gn * 64], F32, tag="if_sb")
    nc.scalar.activation(
        _sub_ap(if_sb, 0, 128, 0, cpk), _sub_ap(ps_A, 0, 128, 1, ps_int),
        AFT.Sigmoid, bias=b_sb[:, 0:1],
    )
    og_sb = post_pool.tile([128, gn * 64], F32, tag="og_sb")
    nc.scalar.activation(
        _sub_ap(og_sb, p_ch, 64, 0, cpk), _sub_ap(ps_B, p_ch, 64, 1, ps_int),
        AFT.Sigmoid, bias=b_sb[p_ch:p_ch + 64, 1:2],
    )
    nc.scalar.activation(
        _sub_ap(og_sb, p_ig, 64, 0, cpk), _sub_ap(ps_B, p_ig, 64, 1, ps_int),
        AFT.Tanh, bias=b_sb[p_ig:p_ig + 64, 1:2],
    )

    # t1 = i * g on the product half, then DMA across to the chain half.
    t1 = post_pool.tile([128, gn * 64], F32, tag="t1")
    nc.vector.tensor_mul(
        _sub_ap(t1, p_ig, 64, 0, [[1, gn * 64]]),
        _sub_ap(if_sb, p_ig, 64, 0, [[1, gn * 64]]),
        _sub_ap(og_sb, p_ig, 64, 0, [[1, gn * 64]]),
    )
    c_ap = _sub_ap(cstate[:], p_ch, 64, gbase, [[1, gn * 64]])
    if first_step:
        # c was zero: c = i * g, moved directly into the state.
        nc.sync.dma_start(c_ap, _sub_ap(t1, p_ig, 64, 0, [[1, gn * 64]]))
    else:
        t1x = post_pool.tile([128, gn * 64], F32, tag="t1x")
        nc.sync.dma_start(
            _sub_ap(t1x, p_ch, 64, 0, [[1, gn * 64]]),
            _sub_ap(t1, p_ig, 64, 0, [[1, gn * 64]]),
        )
        t2 = post_pool.tile([128, gn * 64], F32, tag="t2")
        nc.vector.tensor_mul(
            _sub_ap(t2, p_ch, 64, 0, [[1, gn * 64]]),
            _sub_ap(if_sb, p_ch, 64, 0, [[1, gn * 64]]),
            c_ap,
        )
        nc.vector.tensor_add(
            c_ap,
            _sub_ap(t2, p_ch, 64, 0, [[1, gn * 64]]),
            _sub_ap(t1x, p_ch, 64, 0, [[1, gn * 64]]),
        )

    th = post_pool.tile([128, gn * 64], F32, tag="th")
    th_ap = _sub_ap(th, p_ch, 64, 0, [[1, gn * 64]])
    nc.scalar.activation(th_ap, c_ap, AFT.Tanh)
    o_ap = _sub_ap(og_sb, p_ch, 64, 0, [[64, gn], [1, 64]])
    th_s = _sub_ap(th, p_ch, 64, 0, [[64, gn], [1, 64]])
    for (dest_tile, dest_p) in h_dests:
        dest = _sub_ap(dest_tile[:], dest_p, 64,
                       INT_OFF + cs * WP, [[WP, gn], [1, 64]])
        nc.vector.tensor_mul(dest, o_ap, th_s)


def _zero_pads(nc, comb, zp, p0, n_p):
    """Zero a comb tile's pad columns and guard elements: positions
    {66k, 66k+1 : k=0..64} are exactly the two guards plus every row's
    left/right pad column. Done via DMA from a zeros tensor (memset can't
    write f32r)."""
    nc.sync.dma_start(
        _sub_ap(comb[:], p0, n_p, 0, [[WP, H + 1], [1, 2]]),
        _sub_ap(zp[:], p0, n_p, 0, [[2, H + 1], [1, 2]]).bitcast(F32R),
    )


def build_program(t_steps=T_STEPS):
    nc = bacc.Bacc("TRN2", target_bir_lowering=False, debug=False, num_devices=8)

    xc = nc.dram_tensor("xc", [t_steps * CIN, COMB_N], F32, kind="ExternalInput")
    zp = nc.dram_tensor("zp", [128, 2 * (H + 1)], F32, kind="ExternalInput")
    w0t = nc.dram_tensor("w0t", [K0, 18 * 128], F32, kind="ExternalInput")
    w1t = nc.dram_tensor("w1t", [K1, 18 * 128], F32, kind="ExternalInput")
    b0t = nc.dram_tensor("b0t", [128, 2], F32, kind="ExternalInput")
    b1t = nc.dram_tensor("b1t", [128, 2], F32, kind="ExternalInput")

    out1 = nc.dram_tensor("out1", [t_steps * HD, HW], F32, kind="ExternalOutput")
    h0f = nc.dram_tensor("h0f", [HD, HW], F32, kind="ExternalOutput")
    c0f = nc.dram_tensor("c0f", [HD, HW], F32, kind="ExternalOutput")
    c1f = nc.dram_tensor("c1f", [HD, HW], F32, kind="ExternalOutput")

    x_dst = [[WP, H], [1, W]]  # interior rows of a comb tile
    x_src = [[W, H], [1, W]]

    with TileContext(nc) as tc:
        with (
            tc.tile_pool(name="const", bufs=1) as const_pool,
            tc.tile_pool(name="comb0", bufs=2) as comb0_pool,
            tc.tile_pool(name="comb1", bufs=2) as comb1_pool,
            tc.tile_pool(name="state", bufs=1) as state_pool,
            tc.tile_pool(name="post", bufs=2) as post_pool,
            tc.tile_pool(name="psum", bufs=4, space="PSUM") as psum_pool,
        ):
            # The t=0 conv contracts only x channels (w0 rows 64:80): load
            # those + x(0) + comb0 pads first so the PE starts ASAP.
            w0_sb = const_pool.tile([K0, 18 * 128], F32R, tag="w0")
            nc.sync.dma_start(w0_sb[HD:K0, :],
                              _sub_ap(w0t[:], HD, CIN, 0,
                                      [[1, 18 * 128]]).bitcast(F32R))
            b0_sb = const_pool.tile([128, 2], F32, tag="b0")
            nc.sync.dma_start(b0_sb[:], b0t[:])
            comb0_cur = comb0_pool.tile([K0, COMB_N], F32R, tag="comb0")
            _zero_pads(nc, comb0_cur, zp, 0, HD)
            nc.sync.dma_start(
                _sub_ap(comb0_cur[:], HD, CIN, 0, [[1, COMB_N]]),
                _sub_ap(xc[:], 0, CIN, 0, [[1, COMB_N]]).bitcast(F32R),
            )
            nc.sync.dma_start(w0_sb[0:HD, :],
                              _sub_ap(w0t[:], 0, HD, 0,
                                      [[1, 18 * 128]]).bitcast(F32R))
            w1_sb = const_pool.tile([K1, 18 * 128], F32R, tag="w1")
            nc.sync.dma_start(w1_sb[:], w1t[:].bitcast(F32R))
            b1_sb = const_pool.tile([128, 2], F32, tag="b1")
            nc.sync.dma_start(b1_sb[:], b1t[:])

            # c0 on partitions 0:64, c1 on 64:128 (compact layout)
            cstate = state_pool.tile([128, HW], F32, tag="cstate")

            comb1_cur = comb1_pool.tile([K1, COMB_N], F32R, tag="comb1")
            _zero_pads(nc, comb1_cur, zp, 0, K1)

            for t in range(t_steps):
                comb0_next = comb0_pool.tile([K0, COMB_N], F32R, tag="comb0")
                _zero_pads(nc, comb0_next, zp, 0, HD)
                if t + 1 < t_steps:
                    nc.sync.dma_start(
                        _sub_ap(comb0_next[:], HD, CIN, 0, [[1, COMB_N]]),
                        _sub_ap(xc[:], (t + 1) * CIN, CIN, 0,
                                [[1, COMB_N]]).bitcast(F32R),
                    )
                comb1_next = comb1_pool.tile([K1, COMB_N], F32R, tag="comb1")
                _zero_pads(nc, comb1_next, zp, 0, K1)

                # ---- layer 0 ----
                # h0(-1) == 0: at t=0 contract only over the x channels,
                # which sit at partitions 64:80.
                k_lo0 = HD if t == 0 else 0
                groups0 = []
                _emit_conv(nc, psum_pool, comb0_cur, w0_sb, k_lo0, K0, groups0)
                for (gi, ps_A, ps_B) in groups0:
                    _emit_post(
                        nc, post_pool, 0, gi, ps_A, ps_B, b0_sb, cstate,
                        [(comb0_next, 0), (comb1_cur, 0)],
                        first_step=(t == 0),
                    )

                if t == t_steps - 1:
                    # h0(15)/c0 are final after layer 0's last posts; drain
                    # them to HBM under layer 1's compute.
                    nc.sync.dma_start(
                        _sub_ap(h0f[:], 0, HD, 0, x_src),
                        _sub_ap(comb0_next[:], 0, HD, INT_OFF, x_dst).bitcast(F32),
                    )
                    nc.sync.dma_start(c0f[:], cstate[0:HD, :])

                # ---- layer 1 ----
                k_hi1 = HD if t == 0 else K1  # h1(-1) == 0: skip 64:128
                groups1 = []
                _emit_conv(nc, psum_pool, comb1_cur, w1_sb, 0, k_hi1, groups1)
                for (gi, ps_A, ps_B) in groups1:
                    _emit_post(
                        nc, post_pool, 1, gi, ps_A, ps_B, b1_sb, cstate,
                        [(comb1_next, HD)],
                        first_step=(t == 0),
                    )
                    if t == t_steps - 1:
                        # Stream the last step's outputs per row-group so the
                        # final DMAs drain under the remaining compute.
                        (cs, cn), _ = GROUPS[gi]
                        gn = 2 * cn
                        nc.sync.dma_start(
                            _sub_ap(out1[:], t * HD, HD, cs * W,
                                    [[W, gn], [1, W]]),
                            _sub_ap(comb1_next[:], HD, HD,
                                    INT_OFF + cs * WP,
                                    [[WP, gn], [1, W]]).bitcast(F32),
                        )
                        nc.sync.dma_start(
                            c1f[:, cs * W:(cs + gn) * W],
                            _sub_ap(cstate[:], HD, HD, cs * W,
                                    [[1, gn * W]]),
                        )

                if t < t_steps - 1:
                    nc.sync.dma_start(
                        _sub_ap(out1[:], t * HD, HD, 0, x_src),
                        _sub_ap(comb1_next[:], HD, HD, INT_OFF, x_dst).bitcast(F32),
                    )

                comb0_cur = comb0_next
                comb1_cur = comb1_next


    nc.compile()
    return nc


def _gate_perm(layer):
    """256-entry output-channel order: [tile A quarters, tile B quarters]."""
    (a0, a1), (b0, b1) = GATE_LAYOUT[layer]
    order = []
    for q in (a0, a1, b0, b1):
        order.extend(range(q * 64, (q + 1) * 64))
    return order


def _prep_weights(w, K, layer):
    """w [256, K, 3, 3] -> [K, 18*128] lhsT blocks per (shift, psum tile).

    Output channels are permuted per GATE_LAYOUT. For layer 0 the
    input-channel rows are also permuted to the on-chip comb0 layout
    [h(64) | x(16)] (reference concat order is [x, h])."""
    w = np.asarray(w, np.float32)[_gate_perm(layer)]
    if layer == 0:
        assert K == K0
        perm = list(range(CIN, K0)) + list(range(CIN))
        w = w[:, perm]
    out = np.empty((K, 18, 128), np.float32)
    for s, (dy, dx) in enumerate(SHIFTS):
        for mh in range(2):
            out[:, s * 2 + mh, :] = w[mh * 128:(mh + 1) * 128, :, dy + 1, dx + 1].T
    return np.ascontiguousarray(out.reshape(K, 18 * 128))


def _prep_bias(b, layer):
    """b [256] -> [128, 2]: col 0 = tile A bias, col 1 = tile B bias."""
    bp = np.asarray(b, np.float32)[_gate_perm(layer)]
    return np.ascontiguousarray(bp.reshape(2, 128).T)


_NC_CACHE = {}


def kernel(x, w0, b0, w1, b1):
    from concourse.bass_utils import run_bass_kernel_spmd

    x = np.ascontiguousarray(np.asarray(x), dtype=np.float32)
    B, T = x.shape[0], x.shape[1]
    assert (B, T) == (8, T_STEPS) and x.shape[2:] == (CIN, H, W)

    w0t = _prep_weights(np.asarray(w0, dtype=np.float32), K0, 0)
    w1t = _prep_weights(np.asarray(w1, dtype=np.float32), K1, 1)
    b0t = _prep_bias(b0, 0)
    b1t = _prep_bias(b1, 1)

    xs = np.zeros((B, T * CIN, COMB_N), np.float32)
    xs[:, :, 1:1 + H * WP].reshape(B, T * CIN, H, WP)[:, :, :, 1:1 + W] = (
        x.reshape(B, T * CIN, H, W)
    )
    zp = np.zeros((128, 2 * (H + 1)), np.float32)
    in_maps = [
        {"xc": xs[i], "w0t": w0t, "w1t": w1t, "b0t": b0t, "b1t": b1t, "zp": zp}
        for i in range(B)
    ]

    if "nc" not in _NC_CACHE:
        _NC_CACHE["nc"] = build_program()
    nc = _NC_CACHE["nc"]

    res = run_bass_kernel_spmd(nc, in_maps, core_ids=list(range(8)), trace=False)

    out1 = np.stack(
        [res.results[i]["out1"].reshape(T_STEPS, HD, H, W) for i in range(B)]
    )
    h0 = np.stack([res.results[i]["h0f"].reshape(HD, H, W) for i in range(B)])
    c0 = np.stack([res.results[i]["c0f"].reshape(HD, H, W) for i in range(B)])
    c1 = np.stack([res.results[i]["c1f"].reshape(HD, H, W) for i in range(B)])
    h1 = np.ascontiguousarray(out1[:, -1])
    return out1, h0, c0, h1, c1


# revision 15
# speedup vs baseline: 1.0129x; 1.0026x over previous
"""ConvLSTM (2 layers, T=16, B=8, 64x64, Hd=64) Trainium2 Bass kernel.

Sharding: data-parallel over batch, one image per NeuronCore (8 cores).
Per core, each timestep's 3x3 SAME conv is computed as 9 shifted fp32r
matmuls accumulating in PSUM (channels on partitions, pixels on the free
dim), gates go through ScalarE (sigmoid/tanh with fused per-partition
bias), and the LSTM cell update runs on VectorE. Recurrent state (h
packed with the conv input, c) stays resident in SBUF for all 16 steps.

Matmul APs must be flat (partition + one contiguous free dim), so conv
inputs are stored width-padded: each 64-pixel row occupies 66 slots
(zero pad columns left/right, plus one guard element at each end of the
tile) at offset y*66; a (dy, dx) shift is then a pure element offset and
every matmul reads/writes one contiguous window. Row clipping handles
dy at the image top/bottom. PSUM holds rows in 512-wide banks (7 or 4
padded rows per bank); the elementwise ops read the interior via strided
APs and keep everything else compact.

Compute-engine ops are partition-aligned (lanes), so the cell update must
keep i, g, f, c, o, tanh(c) on one 64-partition range. Gates are permuted
(via host-side weight-column permutation) so layer 0's chain runs on
partitions 0:64 (where its h must land in comb0/comb1) and layer 1's on
64:128; the single remaining cross-half term (i*g) moves with one small
SBUF->SBUF DMA per row-group.

Layouts (partitions x free):
  comb0 [80, 4226]  = h0 (0:64) | x_t (64:80)      (w0 rows permuted to match)
  comb1 [128, 4226] = h0(t) (0:64) | h1(t-1) (64:128)
  cstate [128, 4096] = c0 (0:64) | c1 (64:128)     (compact)
  layer0 PSUM: ps_A = [f|i], ps_B = [o|g];  layer1: ps_A = [i|f], ps_B = [g|o]
"""
import os
import sys

sys.path.insert(0, "/opt/trn_rl_repo")

# The kernel must run on the axon-tunneled NeuronCores; drop a platform
# pin (e.g. JAX_PLATFORMS=cpu meant for the reference) that would mask it.
if "axon" not in os.environ.get("JAX_PLATFORMS", "axon"):
    os.environ.pop("JAX_PLATFORMS", None)

import numpy as np

import concourse.bass as bass
import concourse.mybir as mybir
from concourse import bacc
from concourse.tile import TileContext

F32 = mybir.dt.float32
F32R = mybir.dt.float32r
AFT = mybir.ActivationFunctionType

T_STEPS = 16
H = 64
W = 64
HW = H * W  # 4096
WP = W + 2  # padded row stride (66)
COMB_N = H * WP + 2  # 4226: guard elem + 64 padded rows + guard elem
CIN = 16
HD = 64
K0 = CIN + HD  # 80
K1 = HD + HD  # 128

# Interior element (y, x) of a comb tile lives at 1 + y*WP + 1 + x.
INT_OFF = 2

# (0,0) first: it covers every chunk position unclipped, so start=True
# initializes the full PSUM region before the clipped shifts accumulate.
SHIFTS = [(0, 0), (-1, -1), (-1, 0), (-1, 1), (0, -1), (0, 1), (1, -1), (1, 0), (1, 1)]

# PSUM bank chunks: (start row, rows). 7 padded rows = 462 <= 512 (one
# fp32 bank); the last 8 rows split 4+4 so each group is uniform.
CHUNKS = [(0, 7), (7, 7), (14, 7), (21, 7), (28, 7), (35, 7), (42, 7), (49, 7),
          (56, 4), (60, 4)]
# Post-processing groups: two consecutive chunks share one [128, 1024]
# PSUM tile (2 banks).
GROUPS = [(CHUNKS[2 * i], CHUNKS[2 * i + 1]) for i in range(len(CHUNKS) // 2)]

# Gate quarters of the conv output, in reference order: i, f, o, g.
# Per layer: (out-channel order for PSUM tile A, for tile B) in units of
# 64-channel quarters (0=i, 1=f, 2=o, 3=g).
GATE_LAYOUT = {
    0: ((1, 0), (2, 3)),  # ps_A = [f|i], ps_B = [o|g]; cell chain on 0:64
    1: ((0, 1), (3, 2)),  # ps_A = [i|f], ps_B = [g|o]; cell chain on 64:128
}


def _sub_ap(tile_ap, p0, n_p, off, pattern):
    """AP over `tile_ap`'s tensor: partitions p0:p0+n_p, free pattern at
    element offset `off` (pattern = [[step, count], ...])."""
    pstride = tile_ap.ap[0][0]
    return bass.AP(
        tile_ap.tensor, tile_ap.offset + p0 * pstride + off,
        [[pstride, n_p]] + pattern,
    )


def _emit_conv(nc, psum_pool, comb, w_sb, k_lo, k_hi, psum_out):
    """One layer's 3x3 conv for one timestep: gates[256, :] into PSUM.

    comb: width-padded SBUF tile whose partitions k_lo:k_hi hold the input
    channels to contract over. w_sb: [K, 18*128] with column block
    (s*2+mh)*128 holding the [K, 128] transposed weights of shift s, PSUM
    tile mh (A=0, B=1). psum_out collects (group_idx, ps_A, ps_B).
    """
    n_k = k_hi - k_lo
    for gi, group in enumerate(GROUPS):
        ps_pair = []
        for mh in range(2):
            ps = psum_pool.tile([128, 1024], F32)
            for ci, (cs, cn) in enumerate(group):
                for s, (dy, dx) in enumerate(SHIFTS):
                    ys = max(cs, -dy)
                    ye = min(cs + cn, H - dy)
                    nr = ye - ys
                    rhs = _sub_ap(comb[:], k_lo, n_k,
                                  1 + (ys + dy) * WP + dx, [[1, nr * WP]])
                    out = _sub_ap(ps, 0, 128,
                                  ci * 512 + (ys - cs) * WP, [[1, nr * WP]])
                    lhsT = w_sb[k_lo:k_hi,
                                (s * 2 + mh) * 128:(s * 2 + mh + 1) * 128]
                    nc.tensor.matmul(
                        out, lhsT, rhs,
                        start=(s == 0), stop=(s == len(SHIFTS) - 1),
                    )
            ps_pair.append(ps)
        psum_out.append((gi, ps_pair[0], ps_pair[1]))


def _emit_post(nc, post_pool, layer, gi, ps_A, ps_B, b_sb, cstate, h_dests,
               first_step):
    """LSTM elementwise for one row-group (2 PSUM banks, gn rows).

    layer 0: cell chain on partitions 0:64 (c = cstate[0:64]); the i*g
    product forms on 64:128 and is DMA'd down. layer 1: mirrored.
    h_dests: (tile, partition base) pairs receiving h = o*tanh(c) into
    their padded interior rows.
    """
    (cs, cn), (_, cn2) = GROUPS[gi]
    assert cn == cn2
    gn = 2 * cn  # rows in this group
    gbase = cs * 64  # compact element offset of the group's first row

    lo, hi = 0, 64
    p_ch = lo if layer == 0 else hi  # chain half (f, o, c, th, h)
    p_ig = hi if layer == 0 else lo  # product half (i, g)

    # PSUM interior of the 2-bank group: [2 banks, cn rows, 64 cols]
    ps_int = [[512, 2], [WP, cn], [1, 64]]
    # matching compact layout: [2, cn, 64]
    cpk = [[cn * 64, 2], [64, cn], [1, 64]]

    if_sb = post_pool.tile([128, gn * 64], F32, tag="if_sb")
    nc.scalar.activation(
        _sub_ap(if_sb, 0, 128, 0, cpk), _sub_ap(ps_A, 0, 128, 1, ps_int),
        AFT.Sigmoid, bias=b_sb[:, 0:1],
    )
    og_sb = post_pool.tile([128, gn * 64], F32, tag="og_sb")
    nc.scalar.activation(
        _sub_ap(og_sb, p_ch, 64, 0, cpk), _sub_ap(ps_B, p_ch, 64, 1, ps_int),
        AFT.Sigmoid, bias=b_sb[p_ch:p_ch + 64, 1:2],
    )
    nc.scalar.activation(
        _sub_ap(og_sb, p_ig, 64, 0, cpk), _sub_ap(ps_B, p_ig, 64, 1, ps_int),
        AFT.Tanh, bias=b_sb[p_ig:p_ig + 64, 1:2],
    )

    # t1 = i * g on the product half, then DMA across to the chain half.
    t1 = post_pool.tile([128, gn * 64], F32, tag="t1")
    nc.vector.tensor_mul(
        _sub_ap(t1, p_ig, 64, 0, [[1, gn * 64]]),
        _sub_ap(if_sb, p_ig, 64, 0, [[1, gn * 64]]),
        _sub_ap(og_sb, p_ig, 64, 0, [[1, gn * 64]]),
    )
    c_ap = _sub_ap(cstate[:], p_ch, 64, gbase, [[1, gn * 64]])
    if first_step:
        # c was zero: c = i * g, moved directly into the state.
        nc.sync.dma_start(c_ap, _sub_ap(t1, p_ig, 64, 0, [[1, gn * 64]]))
    else:
        t1x = post_pool.tile([128, gn * 64], F32, tag="t1x")
        nc.sync.dma_start(
            _sub_ap(t1x, p_ch, 64, 0, [[1, gn * 64]]),
            _sub_ap(t1, p_ig, 64, 0, [[1, gn * 64]]),
        )
        t2 = post_pool.tile([128, gn * 64], F32, tag="t2")
        nc.vector.tensor_mul(
            _sub_ap(t2, p_ch, 64, 0, [[1, gn * 64]]),
            _sub_ap(if_sb, p_ch, 64, 0, [[1, gn * 64]]),
            c_ap,
        )
        nc.vector.tensor_add(
            c_ap,
            _sub_ap(t2, p_ch, 64, 0, [[1, gn * 64]]),
            _sub_ap(t1x, p_ch, 64, 0, [[1, gn * 64]]),
        )

    th = post_pool.tile([128, gn * 64], F32, tag="th")
    th_ap = _sub_ap(th, p_ch, 64, 0, [[1, gn * 64]])
    nc.scalar.activation(th_ap, c_ap, AFT.Tanh)
    o_ap = _sub_ap(og_sb, p_ch, 64, 0, [[64, gn], [1, 64]])
    th_s = _sub_ap(th, p_ch, 64, 0, [[64, gn], [1, 64]])
    for (dest_tile, dest_p) in h_dests:
        dest = _sub_ap(dest_tile[:], dest_p, 64,
                       INT_OFF + cs * WP, [[WP, gn], [1, 64]])
        nc.vector.tensor_mul(dest, o_ap, th_s)


def _zero_pads(nc, comb, zp, p0, n_p):
    """Zero a comb tile's pad columns and guard elements: positions
    {66k, 66k+1 : k=0..64} are exactly the two guards plus every row's
    left/right pad column. Done via DMA from a zeros tensor (memset can't
    write f32r)."""
    nc.sync.dma_start(
        _sub_ap(comb[:], p0, n_p, 0, [[WP, H + 1], [1, 2]]),
        _sub_ap(zp[:], p0, n_p, 0, [[2, H + 1], [1, 2]]).bitcast(F32R),
    )


def build_program(t_steps=T_STEPS):
    nc = bacc.Bacc("TRN2", target_bir_lowering=False, debug=False, num_devices=8)

    xc = nc.dram_tensor("xc", [t_steps * CIN, COMB_N], F32, kind="ExternalInput")
    zp = nc.dram_tensor("zp", [128, 2 * (H + 1)], F32, kind="ExternalInput")
    w0t = nc.dram_tensor("w0t", [K0, 18 * 128], F32, kind="ExternalInput")
    w1t = nc.dram_tensor("w1t", [K1, 18 * 128], F32, kind="ExternalInput")
    b0t = nc.dram_tensor("b0t", [128, 2], F32, kind="ExternalInput")
    b1t = nc.dram_tensor("b1t", [128, 2], F32, kind="ExternalInput")

    out1 = nc.dram_tensor("out1", [t_steps * HD, HW], F32, kind="ExternalOutput")
    h0f = nc.dram_tensor("h0f", [HD, HW], F32, kind="ExternalOutput")
    c0f = nc.dram_tensor("c0f", [HD, HW], F32, kind="ExternalOutput")
    c1f = nc.dram_tensor("c1f", [HD, HW], F32, kind="ExternalOutput")

    x_dst = [[WP, H], [1, W]]  # interior rows of a comb tile
    x_src = [[W, H], [1, W]]

    with TileContext(nc) as tc:
        with (
            tc.tile_pool(name="const", bufs=1) as const_pool,
            tc.tile_pool(name="comb0", bufs=2) as comb0_pool,
            tc.tile_pool(name="comb1", bufs=2) as comb1_pool,
            tc.tile_pool(name="state", bufs=1) as state_pool,
            tc.tile_pool(name="post", bufs=2) as post_pool,
            tc.tile_pool(name="psum", bufs=4, space="PSUM") as psum_pool,
        ):
            # The t=0 conv contracts only x channels (w0 rows 64:80): load
            # those + x(0) + comb0 pads first so the PE starts ASAP.
            # t=0's conv reads only the x channels (partitions 64:80), whose
            # pads arrive with the padded x rows themselves; everything else
            # (h-half pads, biases, w1) is needed later and loads behind.
            w0_sb = const_pool.tile([K0, 18 * 128], F32R, tag="w0")
            nc.sync.dma_start(w0_sb[HD:K0, :],
                              _sub_ap(w0t[:], HD, CIN, 0,
                                      [[1, 18 * 128]]).bitcast(F32R))
            comb0_cur = comb0_pool.tile([K0, COMB_N], F32R, tag="comb0")
            nc.sync.dma_start(
                _sub_ap(comb0_cur[:], HD, CIN, 0, [[1, COMB_N]]),
                _sub_ap(xc[:], 0, CIN, 0, [[1, COMB_N]]).bitcast(F32R),
            )
            nc.sync.dma_start(w0_sb[0:HD, :],
                              _sub_ap(w0t[:], 0, HD, 0,
                                      [[1, 18 * 128]]).bitcast(F32R))
            b0_sb = const_pool.tile([128, 2], F32, tag="b0")
            nc.sync.dma_start(b0_sb[:], b0t[:])
            _zero_pads(nc, comb0_cur, zp, 0, HD)
            w1_sb = const_pool.tile([K1, 18 * 128], F32R, tag="w1")
            nc.sync.dma_start(w1_sb[:], w1t[:].bitcast(F32R))
            b1_sb = const_pool.tile([128, 2], F32, tag="b1")
            nc.sync.dma_start(b1_sb[:], b1t[:])

            # c0 on partitions 0:64, c1 on 64:128 (compact layout)
            cstate = state_pool.tile([128, HW], F32, tag="cstate")

            comb1_cur = comb1_pool.tile([K1, COMB_N], F32R, tag="comb1")
            _zero_pads(nc, comb1_cur, zp, 0, K1)

            for t in range(t_steps):
                comb0_next = comb0_pool.tile([K0, COMB_N], F32R, tag="comb0")
                _zero_pads(nc, comb0_next, zp, 0, HD)
                if t + 1 < t_steps:
                    nc.sync.dma_start(
                        _sub_ap(comb0_next[:], HD, CIN, 0, [[1, COMB_N]]),
                        _sub_ap(xc[:], (t + 1) * CIN, CIN, 0,
                                [[1, COMB_N]]).bitcast(F32R),
                    )
                comb1_next = comb1_pool.tile([K1, COMB_N], F32R, tag="comb1")
                _zero_pads(nc, comb1_next, zp, 0, K1)

                # ---- layer 0 ----
                # h0(-1) == 0: at t=0 contract only over the x channels,
                # which sit at partitions 64:80.
                k_lo0 = HD if t == 0 else 0
                groups0 = []
                _emit_conv(nc, psum_pool, comb0_cur, w0_sb, k_lo0, K0, groups0)
                for (gi, ps_A, ps_B) in groups0:
                    _emit_post(
                        nc, post_pool, 0, gi, ps_A, ps_B, b0_sb, cstate,
                        [(comb0_next, 0), (comb1_cur, 0)],
                        first_step=(t == 0),
                    )

                if t == t_steps - 1:
                    # h0(15)/c0 are final after layer 0's last posts; drain
                    # them to HBM under layer 1's compute.
                    nc.sync.dma_start(
                        _sub_ap(h0f[:], 0, HD, 0, x_src),
                        _sub_ap(comb0_next[:], 0, HD, INT_OFF, x_dst).bitcast(F32),
                    )
                    nc.sync.dma_start(c0f[:], cstate[0:HD, :])

                # ---- layer 1 ----
                k_hi1 = HD if t == 0 else K1  # h1(-1) == 0: skip 64:128
                groups1 = []
                _emit_conv(nc, psum_pool, comb1_cur, w1_sb, 0, k_hi1, groups1)
                for (gi, ps_A, ps_B) in groups1:
                    _emit_post(
                        nc, post_pool, 1, gi, ps_A, ps_B, b1_sb, cstate,
                        [(comb1_next, HD)],
                        first_step=(t == 0),
                    )
                    if t == t_steps - 1:
                        # Stream the last step's outputs per row-group so the
                        # final DMAs drain under the remaining compute.
                        (cs, cn), _ = GROUPS[gi]
                        gn = 2 * cn
                        nc.sync.dma_start(
                            _sub_ap(out1[:], t * HD, HD, cs * W,
                                    [[W, gn], [1, W]]),
                            _sub_ap(comb1_next[:], HD, HD,
                                    INT_OFF + cs * WP,
                                    [[WP, gn], [1, W]]).bitcast(F32),
                        )
                        nc.sync.dma_start(
                            c1f[:, cs * W:(cs + gn) * W],
                            _sub_ap(cstate[:], HD, HD, cs * W,
                                    [[1, gn * W]]),
                        )

                if t < t_steps - 1:
                    nc.sync.dma_start(
                        _sub_ap(out1[:], t * HD, HD, 0, x_src),
                        _sub_ap(comb1_next[:], HD, HD, INT_OFF, x_dst).bitcast(F32),
                    )

                comb0_cur = comb0_next
                comb1_cur = comb1_next


    nc.compile()
    return nc


def _gate_perm(layer):
    """256-entry output-channel order: [tile A quarters, tile B quarters]."""
    (a0, a1), (b0, b1) = GATE_LAYOUT[layer]
    order = []
    for q in (a0, a1, b0, b1):
        order.extend(range(q * 64, (q + 1) * 64))
    return order


def _prep_weights(w, K, layer):
    """w [256, K, 3, 3] -> [K, 18*128] lhsT blocks per (shift, psum tile).

    Output channels are permuted per GATE_LAYOUT. For layer 0 the
    input-channel rows are also permuted to the on-chip comb0 layout
    [h(64) | x(16)] (reference concat order is [x, h])."""
    w = np.asarray(w, np.float32)[_gate_perm(layer)]
    if layer == 0:
        assert K == K0
        perm = list(range(CIN, K0)) + list(range(CIN))
        w = w[:, perm]
    out = np.empty((K, 18, 128), np.float32)
    for s, (dy, dx) in enumerate(SHIFTS):
        for mh in range(2):
            out[:, s * 2 + mh, :] = w[mh * 128:(mh + 1) * 128, :, dy + 1, dx + 1].T
    return np.ascontiguousarray(out.reshape(K, 18 * 128))


def _prep_bias(b, layer):
    """b [256] -> [128, 2]: col 0 = tile A bias, col 1 = tile B bias."""
    bp = np.asarray(b, np.float32)[_gate_perm(layer)]
    return np.ascontiguousarray(bp.reshape(2, 128).T)


_NC_CACHE = {}


def kernel(x, w0, b0, w1, b1):
    from concourse.bass_utils import run_bass_kernel_spmd

    x = np.ascontiguousarray(np.asarray(x), dtype=np.float32)
    B, T = x.shape[0], x.shape[1]
    assert (B, T) == (8, T_STEPS) and x.shape[2:] == (CIN, H, W)

    w0t = _prep_weights(np.asarray(w0, dtype=np.float32), K0, 0)
    w1t = _prep_weights(np.asarray(w1, dtype=np.float32), K1, 1)
    b0t = _prep_bias(b0, 0)
    b1t = _prep_bias(b1, 1)

    xs = np.zeros((B, T * CIN, COMB_N), np.float32)
    xs[:, :, 1:1 + H * WP].reshape(B, T * CIN, H, WP)[:, :, :, 1:1 + W] = (
        x.reshape(B, T * CIN, H, W)
    )
    zp = np.zeros((128, 2 * (H + 1)), np.float32)
    in_maps = [
        {"xc": xs[i], "w0t": w0t, "w1t": w1t, "b0t": b0t, "b1t": b1t, "zp": zp}
        for i in range(B)
    ]

    if "nc" not in _NC_CACHE:
        _NC_CACHE["nc"] = build_program()
    nc = _NC_CACHE["nc"]

    res = run_bass_kernel_spmd(nc, in_maps, core_ids=list(range(8)), trace=False)

    out1 = np.stack(
        [res.results[i]["out1"].reshape(T_STEPS, HD, H, W) for i in range(B)]
    )
    h0 = np.stack([res.results[i]["h0f"].reshape(HD, H, W) for i in range(B)])
    c0 = np.stack([res.results[i]["c0f"].reshape(HD, H, W) for i in range(B)])
    c1 = np.stack([res.results[i]["c1f"].reshape(HD, H, W) for i in range(B)])
    h1 = np.ascontiguousarray(out1[:, -1])
    return out1, h0, c0, h1, c1
